# revision 27
# baseline (speedup 1.0000x reference)
"""Trainium2 Bass kernel for nn_Attention_76450417868987.

Module: three Bahdanau-style additive attentions + gated fusion.
Sharding: pure data-parallel, batch 512 -> 64 per core across 8 cores.

v3 design (per core, heavy tensors host-cast to bf16):
  - Big tensors stream in natural layout [(b n), d] as bf16 wide tiles
    [128, A, 512] via three DMA rings: p_* on sync (HWDGE), feats on
    scalar (HWDGE), indicator/weight constants on gpsimd (SWDGE).
  - X = p + h_proj broadcast built on PE in PSUM [128, 1024] (2 banks):
    identity MM copies p; the two K=64 indicator MMs of a tile pair
    row-pack into disjoint PE row groups (ind64/hp duplicated into both
    partition halves) and run concurrently.
  - tanh on ScalarE over [128, 1024] PSUM -> bf16 SBUF (PSUM-source
    ACT is fast, ~225ns/op).
  - score col = ONE fused DVE scalar_tensor_tensor per tile:
    prod = (tt * 1.0) * awb, accum_out = row-sum -> scol_all [128, nt]
    f32, entirely on chip in flat layout. (tensor_tensor_reduce is not
    supported by this walrus build; scalar_tensor_tensor is.)
  - softmax WITHOUT max-subtraction (scores bounded by |aw|_1 ~ 11) and
    WITHOUT the scalar score bias (softmax-invariant). exp -> e_all.
  - block-diag einsum lhsT built in ONE DVE tensor_tensor per branch:
    lt = indT * broadcast(e) via a stride-0 view (DVE ops have a large
    per-op floor; avoid many small ops). No DRAM bounce.
  - einsum: per tile MM [128,64]x[128,512] accumulated into res PSUM,
    plus an N=1 MM with a ones column accumulating the softmax
    denominator (rides the already-loaded weights).
  - res normalized by reciprocal(den) at the end (DVE).
  - gate: batched PE transposes of cont/senti -> one PSUM tile + one
    copy, 20 bf16 matmuls + bias matmul, tanh, fused alpha-dot via
    scalar_tensor_tensor, sigmoid, blend.
"""

import os
import sys

if "/opt/trn_rl_repo" not in sys.path:
    sys.path.insert(0, "/opt/trn_rl_repo")

import numpy as np

B = 512
NA, NCP, NSW = 196, 50, 50
D = 512
M = 8
BL = B // M  # 64
NT_A = BL * NA // 128  # 98
NT_C = BL * NCP // 128  # 25
P = 128
A_A = 7  # wide-group size, att branch (98 = 14*7)
A_C = 5  # wide-group size, cpt/sw branches (25 = 5*5)

_CACHE = {}


def _segs(per_n):
    """Per flattened tile t of [BL*per_n, D]: list of (row_off, run_len, b)."""
    segs = []
    for t in range(BL * per_n // 128):
        lst = []
        r = t * 128
        while r < t * 128 + 128:
            b = r // per_n
            e = min((b + 1) * per_n, t * 128 + 128)
            lst.append((r - t * 128, e - r, b))
            r = e
        segs.append(lst)
    return segs


def _ind64_const(per_n):
    """[64, nt*128] bf16: ind[b, t*128+r] = 1 iff flat row t*128+r in batch b."""
    import ml_dtypes

    segs = _segs(per_n)
    nt = len(segs)
    a = np.zeros((BL, nt * 128), np.float32)
    for t, lst in enumerate(segs):
        for (off, ln, b) in lst:
            a[b, t * 128 + off : t * 128 + off + ln] = 1.0
    return a.astype(ml_dtypes.bfloat16)


def _indT_const(per_n):
    """[128, nt*64] bf16: indT[r, t*64+b] = 1 iff flat row t*128+r in batch b."""
    import ml_dtypes

    segs = _segs(per_n)
    nt = len(segs)
    a = np.zeros((P, nt * BL), np.float32)
    for t, lst in enumerate(segs):
        for (off, ln, b) in lst:
            a[off : off + ln, t * BL + b] = 1.0
    return a.astype(ml_dtypes.bfloat16)


def _build(nc, reps=1, mode="full"):
    import concourse.bass as bass  # noqa: F401
    from concourse import mybir
    from concourse.tile import TileContext

    f32 = mybir.dt.float32
    bf16 = mybir.dt.bfloat16
    AF = mybir.ActivationFunctionType
    OP = mybir.AluOpType
    AX = mybir.AxisListType

    def dpf(name, shape):
        return nc.declare_dram_parameter(name, shape, f32, isOutput=False)

    def dpb(name, shape):
        return nc.declare_dram_parameter(name, shape, bf16, isOutput=False)

    h_d = dpb("h", [BL, D])
    att_f = dpb("att_feats", [BL * NA, D])
    p_att = dpb("p_att_feats", [BL * NA, D])
    cpt_f = dpb("cpt_feats", [BL * NCP, D])
    p_cpt = dpb("p_cpt_feats", [BL * NCP, D])
    senti_d = dpf("senti_feats", [BL, D])
    sw_f = dpb("senti_word_feats", [BL * NSW, D])
    p_sw = dpb("p_senti_word_feats", [BL * NSW, D])

    w_h2att = dpb("c_h2att_w", [D, D])
    b_h2att = dpf("c_h2att_b", [1, D])
    w_h2cpt = dpb("c_h2cpt_w", [D, D])
    b_h2cpt = dpf("c_h2cpt_b", [1, D])
    aw_att_d = dpf("c_attA_w", [1, D])
    aw_cpt_d = dpf("c_cptA_w", [1, D])
    w_h2sw = dpb("s_h2word_w", [D, D])
    b_h2sw = dpf("s_h2word_b", [1, D])
    aw_sw_d = dpf("s_wordA_w", [1, D])
    w_th = dpb("t_h2att_w", [D, D])
    b_th = dpf("t_h2att_b", [1, D])
    w_tc = dpb("t_cont_w", [2 * D, D])
    b_tc = dpf("t_cont_b", [1, D])
    w_ts = dpb("t_senti_w", [2 * D, D])
    b_ts = dpf("t_senti_b", [1, D])
    w_ta_d = dpf("t_alpha_w", [1, D])
    b_ta_d = dpf("t_alpha_b", [1, 1])

    identbf_d = dpb("identbf", [P, P])
    ones4_d = dpf("ones4", [4, P])
    onescol_d = dpb("onescol", [P, 1])
    # ind64 duplicated into both partition halves so pairs of K=64
    # indicator matmuls can row-pack into disjoint PE row groups
    ind64_att_d = dpb("ind64_att", [P, NT_A * 128])
    ind64_50_d = dpb("ind64_50", [P, NT_C * 128])
    indT_att_d = dpb("indT_att", [P, NT_A * BL])
    indT_50_d = dpb("indT_50", [P, NT_C * BL])

    out_d = nc.declare_dram_parameter("out", [BL, 2 * D], f32, isOutput=True)

    with TileContext(nc) as tc:
        with (
            tc.tile_pool(name="const", bufs=1) as constp,
            tc.tile_pool(name="pio", bufs=2) as piop,
            tc.tile_pool(name="fio", bufs=2) as fiop,
            tc.tile_pool(name="work", bufs=2) as workp,
            tc.tile_pool(name="tanh", bufs=3) as tanhp,
            tc.tile_pool(name="small", bufs=2) as smallp,
            tc.tile_pool(name="psx", bufs=2, space="PSUM") as psxp,
            tc.tile_pool(name="psres", bufs=2, space="PSUM") as psresp,
            tc.tile_pool(name="psden", bufs=1, space="PSUM") as psdenp,
            tc.tile_pool(name="psbf", bufs=1, space="PSUM") as psbfp,
        ):
            for _rep in range(reps):
                # ---------------- setup ----------------
                identbf = constp.tile([P, P], bf16, tag="identbf")
                nc.sync.dma_start(identbf[:], identbf_d[:])
                ones4 = constp.tile([4, P], f32, tag="ones4")
                nc.sync.dma_start(ones4[:], ones4_d[:])
                onescol = constp.tile([P, 1], bf16, tag="onescol")
                nc.sync.dma_start(onescol[:], onescol_d[:])
                h_sb = constp.tile([BL, D], bf16, tag="h_sb")
                nc.sync.dma_start(h_sb[:], h_d[:])
                ind64_a = constp.tile([P, NT_A * 128], bf16, tag="ind64_a")
                nc.gpsimd.dma_start(ind64_a[:], ind64_att_d[:])
                ind64_5 = constp.tile([P, NT_C * 128], bf16, tag="ind64_5")
                nc.gpsimd.dma_start(ind64_5[:], ind64_50_d[:])
                indT_a = constp.tile([P, NT_A * BL], bf16, tag="indT_a")
                nc.gpsimd.dma_start(indT_a[:], indT_att_d[:])
                indT_5 = constp.tile([P, NT_C * BL], bf16, tag="indT_5")
                nc.gpsimd.dma_start(indT_5[:], indT_50_d[:])

                # hT[:, c, :] = h[:, 128c:128(c+1)].T  (PE transposes batched
                # into one PSUM tile, single copy out)
                hT = constp.tile([P, 4, BL], bf16, tag="hT")
                tp4 = psbfp.tile([P, 8, BL], bf16, tag="tpbf")
                for c in range(4):
                    nc.tensor.transpose(
                        tp4[:, c, :], h_sb[:, c * P : (c + 1) * P], identbf[:BL, :BL]
                    )
                nc.scalar.copy(hT[:], tp4[:, :4, :])

                def bcast_row(dram_row, tag, dtype):
                    """-> sbuf [128, D] with every partition = the dram row."""
                    row = smallp.tile([1, D], f32, tag="brow")
                    nc.sync.dma_start(row[:], dram_row[:1, :])
                    ps = psxp.tile([P, 2 * D], f32, tag="xps")
                    nc.tensor.matmul(
                        ps[:, :D], ones4[:1, :], row[:], start=True, stop=True
                    )
                    sb = constp.tile([P, D], dtype, tag=tag)
                    nc.scalar.copy(sb[:], ps[:, :D])
                    return sb

                awb = {
                    "a": bcast_row(aw_att_d, "awb_a", bf16),
                    "c": bcast_row(aw_cpt_d, "awb_c", bf16),
                    "s": bcast_row(aw_sw_d, "awb_s", bf16),
                }
                alphab = bcast_row(w_ta_d, "alphab", f32)

                ab_sb = smallp.tile([1, 1], f32, tag="ab_sb")
                nc.sync.dma_start(ab_sb[:], b_ta_d[:])
                ps = psxp.tile([P, 2 * D], f32, tag="xps")
                nc.tensor.matmul(
                    ps[:BL, :1], ones4[:1, :BL], ab_sb[:], start=True, stop=True
                )
                ab_col = constp.tile([BL, 1], f32, tag="ab_col")
                nc.scalar.copy(ab_col[:], ps[:BL, :1])

                def proj(wd, bd, tag):
                    """hp = h @ W + b -> sbuf [64, 512] bf16."""
                    hp_ps = psxp.tile([P, 2 * D], f32, tag="xps")
                    wt4 = constp.tile([P, 4, D], bf16, tag=f"w_{tag}")
                    nc.gpsimd.dma_start(
                        wt4[:], wd[:].rearrange("(c p) d -> p c d", p=128)
                    )
                    for c in range(4):
                        nc.tensor.matmul(
                            hp_ps[:BL, :D], hT[:, c, :], wt4[:, c, :],
                            start=(c == 0), stop=False,
                        )
                    brow = smallp.tile([1, D], f32, tag="brow")
                    nc.sync.dma_start(brow[:], bd[:1, :])
                    nc.tensor.matmul(
                        hp_ps[:BL, :D], ones4[:1, :BL], brow[:], start=False, stop=True
                    )
                    # duplicated into both partition halves for row-packed MMs
                    sb = constp.tile([P, D], bf16, tag=tag)
                    nc.scalar.copy(sb[:BL, :], hp_ps[:BL, :D])
                    nc.scalar.copy(sb[BL:, :], hp_ps[:BL, :D])
                    return sb

                hp = {
                    "a": proj(w_h2att, b_h2att, "hp_a"),
                    "c": proj(w_h2cpt, b_h2cpt, "hp_c"),
                    "s": proj(w_h2sw, b_h2sw, "hp_s"),
                }

                # scol_all / e_all: flat scores for all 3 branches
                # columns [0:98]=a, [98:123]=c, [123:148]=s
                NT_ALL = NT_A + 2 * NT_C
                scol_all = constp.tile([P, NT_ALL], f32, tag="scol_all")
                e_all = constp.tile([P, NT_ALL], f32, tag="e_all")
                COFF = {"a": 0, "c": NT_A, "s": NT_A + NT_C}

                # ---------------- score phase ----------------
                def score_branch(key, p_dram, nt, ind_sb):
                    Ag = A_A if nt == NT_A else A_C
                    for g in range(nt // Ag):
                        wide7 = piop.tile([P, A_A, D], bf16, tag="p_in")
                        wide = wide7[:, :Ag, :]
                        view = p_dram[g * Ag * 128 : (g + 1) * Ag * 128, :].rearrange(
                            "(a p) d -> p a d", p=128
                        )
                        nc.sync.dma_start(wide, view)
                        if mode == "dma":
                            continue
                        # process pairs of tiles -> one [128, 1024] PSUM tile.
                        # identity MMs use the full array (K=128); the two
                        # K=64 indicator MMs row-pack into disjoint row
                        # groups (partitions 0:64 / 64:128) and run
                        # concurrently on the PE.
                        pairs = [(i, min(i + 2, Ag)) for i in range(0, Ag, 2)]
                        for (i0, i1) in pairs:
                            na = i1 - i0
                            xps = psxp.tile([P, 2 * D], f32, tag="xps")
                            for a in range(i0, i1):
                                col = (a - i0) * D
                                nc.tensor.matmul(
                                    xps[:, col : col + D],
                                    identbf[:],
                                    wide[:, a, :],
                                    start=True,
                                    stop=False,
                                )
                            for a in range(i0, i1):
                                t = g * Ag + a
                                col = (a - i0) * D
                                rg = (a - i0) * BL  # row group 0 or 64
                                nc.tensor.matmul(
                                    xps[:, col : col + D],
                                    ind_sb[rg : rg + BL, t * 128 : (t + 1) * 128],
                                    hp[key][rg : rg + BL, :],
                                    start=False,
                                    stop=True,
                                )
                            if mode == "pe":
                                continue
                            tt = tanhp.tile([P, 2 * D], bf16, tag="tt")
                            nc.scalar.activation(
                                tt[:, : na * D], xps[:, : na * D], AF.Tanh
                            )
                            if mode == "act":
                                continue
                            prod = tanhp.tile([P, 2 * D], bf16, tag="prod")
                            for a in range(i0, i1):
                                t = g * Ag + a
                                col = (a - i0) * D
                                # fused: prod = tt * awb, scol = sum(prod)
                                nc.vector.scalar_tensor_tensor(
                                    prod[:, col : col + D],
                                    tt[:, col : col + D],
                                    1.0,
                                    awb[key][:],
                                    OP.mult,
                                    OP.mult,
                                    accum_out=scol_all[
                                        :, COFF[key] + t : COFF[key] + t + 1
                                    ],
                                )

                def build_lhst(key, nt, indT_sb):
                    """exp of scores -> block-diag lhsT stack [128, nt*64] bf16."""
                    c0 = COFF[key]
                    lt = constp.tile([P, nt * BL], bf16, tag=f"lt_{key}")
                    if mode in ("dma", "pe", "act"):
                        return indT_sb  # use raw indicator so einsum still runs
                    nc.scalar.activation(
                        e_all[:, c0 : c0 + nt], scol_all[:, c0 : c0 + nt], AF.Exp
                    )
                    # ONE broadcast tensor_tensor per branch: lt[r, (t,b)] =
                    # indT[r, (t,b)] * e[r, t] via a stride-0 view of e.
                    e_bc = (
                        e_all[:, c0 : c0 + nt]
                        .rearrange("p t -> p t ()")
                        .broadcast_to((P, nt, BL))
                    )
                    nc.vector.tensor_tensor(
                        lt[:].rearrange("p (t b) -> p t b", b=BL),
                        indT_sb[:].rearrange("p (t b) -> p t b", b=BL),
                        e_bc,
                        op=OP.mult,
                    )
                    return lt

                # ---------------- einsum phase ----------------
                den_all = psdenp.tile([BL, 16], f32, tag="den")
                DCOL = {"a": 0, "c": 4, "s": 8}

                def einsum_branch(key, f_dram, lt, nt):
                    res = psresp.tile([BL, D], f32, tag="res")
                    dc = DCOL[key]
                    Ag = A_A if nt == NT_A else A_C
                    for g in range(nt // Ag):
                        wide7 = fiop.tile([P, A_A, D], bf16, tag="f_in")
                        wide = wide7[:, :Ag, :]
                        view = f_dram[g * Ag * 128 : (g + 1) * Ag * 128, :].rearrange(
                            "(a p) d -> p a d", p=128
                        )
                        nc.scalar.dma_start(wide, view)
                        if mode == "dma":
                            continue
                        for a in range(Ag):
                            t = g * Ag + a
                            nc.tensor.matmul(
                                res[:],
                                lt[:, t * BL : (t + 1) * BL],
                                wide[:, a, :],
                                start=(t == 0),
                                stop=(t == nt - 1),
                            )
                            # denominator rides the already-loaded weights
                            nc.tensor.matmul(
                                den_all[:, dc : dc + 1],
                                lt[:, t * BL : (t + 1) * BL],
                                onescol[:],
                                start=(t == 0),
                                stop=(t == nt - 1),
                            )
                    return res

                cont = constp.tile([BL, 2 * D], f32, tag="cont")
                sent = constp.tile([BL, 2 * D], f32, tag="sent")
                nc.sync.dma_start(sent[:, :D], senti_d[:])

                full = mode == "full"

                def norm_res(key, res, dst):
                    """dst = res / den (softmax denominator)."""
                    dc = DCOL[key]
                    rec = smallp.tile([BL, 1], f32, tag=f"rec_{key}")
                    nc.vector.reciprocal(rec[:], den_all[:, dc : dc + 1])
                    nc.vector.tensor_scalar_mul(dst, res[:], rec[:])

                # emission order tuned so each branch's exp/lhsT-build lands
                # in the ACT/DVE queues right after that branch's tanh/stt
                # ops (no FIFO stall behind the next branch), while score(c)
                # keeps the PE busy while lhsT(a) is being built.
                score_branch("a", p_att, NT_A, ind64_a)
                lt_a = build_lhst("a", NT_A, indT_a)
                score_branch("c", p_cpt, NT_C, ind64_5)
                res_a = einsum_branch("a", att_f, lt_a, NT_A)
                if full:
                    norm_res("a", res_a, cont[:, :D])
                lt_c = build_lhst("c", NT_C, indT_5)
                score_branch("s", p_sw, NT_C, ind64_5)
                res_c = einsum_branch("c", cpt_f, lt_c, NT_C)
                if full:
                    norm_res("c", res_c, cont[:, D:])
                lt_s = build_lhst("s", NT_C, indT_5)
                res_s = einsum_branch("s", sw_f, lt_s, NT_C)
                if full:
                    norm_res("s", res_s, sent[:, D:])

                if not full:
                    fin0 = constp.tile([BL, 2 * D], f32, tag="fin")
                    nc.vector.memset(fin0[:], 0.0)
                    nc.sync.dma_start(out_d[:], fin0[:])
                    continue

                # ---------------- gate ----------------
                # cast cont/sent to bf16, batch 8 PE transposes into one PSUM
                # tile, single ACT copy out -> lhsT chunks [128, 8, 64]
                def gate_lhsT(src_sb, tag):
                    cb = workp.tile([BL, 2 * D], bf16, tag="gcast")
                    nc.vector.tensor_copy(cb[:], src_sb[:])
                    tp = psbfp.tile([P, 8, BL], bf16, tag="tpbf")
                    for c in range(8):
                        nc.tensor.transpose(
                            tp[:, c, :], cb[:, c * P : (c + 1) * P],
                            identbf[:BL, :BL],
                        )
                    ct = constp.tile([P, 8, BL], bf16, tag=tag)
                    nc.scalar.copy(ct[:], tp[:])
                    return ct

                contT = gate_lhsT(cont, "contT")
                sentT = gate_lhsT(sent, "sentT")

                wgtc = constp.tile([P, 8, D], bf16, tag="wgtc")
                nc.gpsimd.dma_start(
                    wgtc[:], w_tc[:].rearrange("(c p) d -> p c d", p=128)
                )
                wgts = constp.tile([P, 8, D], bf16, tag="wgts")
                nc.gpsimd.dma_start(
                    wgts[:], w_ts[:].rearrange("(c p) d -> p c d", p=128)
                )
                wgth = constp.tile([P, 4, D], bf16, tag="wgth")
                nc.gpsimd.dma_start(
                    wgth[:], w_th[:].rearrange("(c p) d -> p c d", p=128)
                )
                g_ps = psresp.tile([BL, D], f32, tag="res")
                first = True
                for (src_t, wt4, nch) in (
                    (contT, wgtc, 8),
                    (sentT, wgts, 8),
                    (hT, wgth, 4),
                ):
                    for c in range(nch):
                        nc.tensor.matmul(
                            g_ps[:], src_t[:, c, :], wt4[:, c, :], start=first,
                            stop=False,
                        )
                        first = False
                b3 = smallp.tile([3, D], f32, tag="b3")
                nc.sync.dma_start(b3[0:1, :], b_tc[:1, :])
                nc.sync.dma_start(b3[1:2, :], b_ts[:1, :])
                nc.sync.dma_start(b3[2:3, :], b_th[:1, :])
                nc.tensor.matmul(
                    g_ps[:], ones4[:3, :BL], b3[:], start=False, stop=True
                )

                g_sb = workp.tile([BL, D], f32, tag="g_sb")
                nc.scalar.activation(g_sb[:], g_ps[:], AF.Tanh)
                gdump = workp.tile([BL, D], f32, tag="gdump")
                gacc = smallp.tile([BL, 1], f32, tag="gacc")
                nc.vector.scalar_tensor_tensor(
                    gdump[:], g_sb[:], 1.0, alphab[:BL, :], OP.mult, OP.mult,
                    accum_out=gacc[:],
                )
                gate = smallp.tile([BL, 1], f32, tag="gate")
                nc.scalar.activation(gate[:], gacc[:], AF.Sigmoid, bias=ab_col[:])

                diff = constp.tile([BL, 2 * D], f32, tag="diff")
                nc.vector.tensor_sub(diff[:], cont[:], sent[:])
                prd = constp.tile([BL, 2 * D], f32, tag="prd")
                nc.vector.tensor_scalar_mul(prd[:], diff[:], gate[:, 0:1])
                fin = constp.tile([BL, 2 * D], f32, tag="fin")
                nc.vector.tensor_add(fin[:], sent[:], prd[:])
                nc.sync.dma_start(out_d[:], fin[:])

    return nc


def _fixup_multiwait(nc):
    """This walrus build allows only ONE sync wait per instruction (except
    InstEventSemaphore). Split extra waits onto same-engine NOPs in front."""
    from concourse import mybir

    nfix = 0
    for fn in nc.m.functions:
        for blk in fn.blocks:
            new = []
            for inst in blk.instructions:
                si = inst.sync_info
                waits = list(si.on_wait) if si is not None else []
                if len(waits) > 1 and type(inst).__name__ != "InstEventSemaphore":
                    for w in waits[:-1]:
                        nop = mybir.InstNoOp(
                            name=nc.get_next_instruction_name(), ins=[], outs=[]
                        )
                        nop.engine = inst.engine
                        nop.sync_info = mybir.SyncInfo(on_wait=[w], on_update=[])
                        nc.register_instruction(nop)
                        new.append(nop)
                        nfix += 1
                    si.on_wait = waits[-1:]
                new.append(inst)
            blk.instructions[:] = new
    return nfix


def _get_nc(reps=1, mode="full"):
    key = f"nc{reps}_{mode}"
    if key not in _CACHE:
        import concourse.bass as bass

        nc = bass.Bass()
        _build(nc, reps=reps, mode=mode)
        nc.finalize()
        _fixup_multiwait(nc)
        _CACHE[key] = nc
    return _CACHE[key]


def _make_in_maps(inputs):
    import ml_dtypes

    bf = ml_dtypes.bfloat16
    f = lambda x: np.ascontiguousarray(np.asarray(x), dtype=np.float32)
    fb = lambda x: np.ascontiguousarray(
        np.asarray(x, dtype=np.float32).astype(bf)
    )

    consts = {
        "identbf": np.eye(P, dtype=np.float32).astype(bf),
        "onescol": np.ones((P, 1), np.float32).astype(bf),
        "ones4": np.ones((4, P), np.float32),
        # duplicated into both partition halves for row-packed PE MMs
        "ind64_att": np.concatenate([_ind64_const(NA)] * 2, axis=0),
        "ind64_50": np.concatenate([_ind64_const(NCP)] * 2, axis=0),
        "indT_att": _indT_const(NA),
        "indT_50": _indT_const(NCP),
    }
    weights = {
        "c_h2att_w": fb(inputs["c_h2att_w"]),
        "c_h2att_b": f(inputs["c_h2att_b"]).reshape(1, D),
        "c_h2cpt_w": fb(inputs["c_h2cpt_w"]),
        "c_h2cpt_b": f(inputs["c_h2cpt_b"]).reshape(1, D),
        "c_attA_w": f(inputs["c_attA_w"]).reshape(1, D),
        "c_cptA_w": f(inputs["c_cptA_w"]).reshape(1, D),
        "s_h2word_w": fb(inputs["s_h2word_w"]),
        "s_h2word_b": f(inputs["s_h2word_b"]).reshape(1, D),
        "s_wordA_w": f(inputs["s_wordA_w"]).reshape(1, D),
        "t_h2att_w": fb(inputs["t_h2att_w"]),
        "t_h2att_b": f(inputs["t_h2att_b"]).reshape(1, D),
        "t_cont_w": fb(inputs["t_cont_w"]),
        "t_cont_b": f(inputs["t_cont_b"]).reshape(1, D),
        "t_senti_w": fb(inputs["t_senti_w"]),
        "t_senti_b": f(inputs["t_senti_b"]).reshape(1, D),
        "t_alpha_w": f(inputs["t_alpha_w"]).reshape(1, D),
        "t_alpha_b": f(inputs["t_alpha_b"]).reshape(1, 1),
    }
    in_maps = []
    for i in range(M):
        sl = slice(i * BL, (i + 1) * BL)
        m = {
            "h": fb(inputs["h"][sl]),
            "att_feats": fb(inputs["att_feats"][sl]).reshape(BL * NA, D),
            "p_att_feats": fb(inputs["p_att_feats"][sl]).reshape(BL * NA, D),
            "cpt_feats": fb(inputs["cpt_feats"][sl]).reshape(BL * NCP, D),
            "p_cpt_feats": fb(inputs["p_cpt_feats"][sl]).reshape(BL * NCP, D),
            "senti_feats": f(inputs["senti_feats"][sl]),
            "senti_word_feats": fb(inputs["senti_word_feats"][sl]).reshape(
                BL * NSW, D
            ),
            "p_senti_word_feats": fb(inputs["p_senti_word_feats"][sl]).reshape(
                BL * NSW, D
            ),
        }
        m.update(weights)
        m.update(consts)
        in_maps.append(m)
    return in_maps


def _run(inputs, trace=False):
    from concourse.bass_utils import run_bass_kernel_spmd

    nc = _get_nc()
    in_maps = _make_in_maps(inputs)
    r = run_bass_kernel_spmd(nc, in_maps, core_ids=list(range(M)), trace=trace)
    out = np.concatenate([r.results[i]["out"] for i in range(M)], axis=0)
    return out, r


def kernel(**inputs):
    out, _ = _run(inputs, trace=False)
    return out


def _timed_runner_make(nc, in_maps, iters):
    """Build a runner for nc with device-resident inputs and pre-staged
    donated output buffers; returns run(i) -> (wall_ns, out_np)."""
    import time

    import jax
    from jax.sharding import Mesh, NamedSharding, PartitionSpec

    try:
        from jax.experimental.shard_map import shard_map
    except ImportError:
        from jax.shard_map import shard_map

    from concourse import bass2jax, mybir
    from concourse.bass2jax import _bass_exec_p

    bass2jax.install_neuronx_cc_hook()
    partition_name = nc.partition_id_tensor.name if nc.partition_id_tensor else None

    in_names, out_names, out_avals, zero_outs = [], [], [], []
    for alloc in nc.m.functions[0].allocations:
        if not isinstance(alloc, mybir.MemoryLocationSet):
            continue
        name = alloc.memorylocations[0].name
        if alloc.kind == "ExternalInput":
            if name != partition_name:
                in_names.append(name)
        elif alloc.kind == "ExternalOutput":
            out_names.append(name)
            out_avals.append(
                jax.core.ShapedArray(
                    tuple(alloc.tensor_shape), mybir.dt.np(alloc.dtype)
                )
            )
            zero_outs.append(
                np.zeros(tuple(alloc.tensor_shape), mybir.dt.np(alloc.dtype))
            )
    n_params = len(in_names)
    n_outs = len(out_names)
    all_in = list(in_names) + list(out_names)
    if partition_name:
        all_in.append(partition_name)

    def _body(*args):
        operands = list(args)
        if partition_name:
            operands.append(bass2jax.partition_id_tensor())
        return tuple(
            _bass_exec_p.bind(
                *operands,
                out_avals=tuple(out_avals),
                in_names=tuple(all_in),
                out_names=tuple(out_names),
                lowering_input_output_aliases=(),
                sim_require_finite=False,
                sim_require_nnan=False,
                nc=nc,
            )
        )

    devices = jax.devices()[:M]
    mesh = Mesh(np.asarray(devices), ("core",))
    donate = tuple(range(n_params, n_params + n_outs))
    sharded = jax.jit(
        shard_map(
            _body,
            mesh=mesh,
            in_specs=(PartitionSpec("core"),) * (n_params + n_outs),
            out_specs=(PartitionSpec("core"),) * n_outs,
            check_rep=False,
        ),
        donate_argnums=donate,
        keep_unused=True,
    )
    sh = NamedSharding(mesh, PartitionSpec("core"))
    per_core = [[np.asarray(m[name]) for name in in_names] for m in in_maps]
    args = [
        jax.device_put(
            np.concatenate([per_core[c][i] for c in range(M)], axis=0), sh
        )
        for i in range(n_params)
    ]
    # one donated zero-output set per call, staged up front
    zsets = []
    for _ in range(iters + 1):
        zsets.append(
            [jax.device_put(np.concatenate([z] * M, axis=0), sh) for z in zero_outs]
        )
    out = sharded(*args, *zsets[-1])
    jax.block_until_ready(out)

    def run(i):
        t0 = time.perf_counter()
        o = sharded(*args, *zsets[i])
        jax.block_until_ready(o)
        t1 = time.perf_counter()
        return (t1 - t0) * 1e9, np.asarray(o[0])

    return run


def profile(inputs, iters=14, mode="full", hi_reps=16):
    """Interleave V1/V<hi> executions; per-pair diffs cancel slow drift of
    the axon dispatch round-trip: T = median(w_hi_i - w1_i) / (hi-1)."""
    in_maps = _make_in_maps(inputs)
    r1 = _timed_runner_make(_get_nc(1, mode), in_maps, iters)
    rh = _timed_runner_make(_get_nc(hi_reps, mode), in_maps, iters)
    w1, wh = [], []
    out = None
    for i in range(iters):
        t, out = r1(i)
        w1.append(t)
        t, _ = rh(i)
        wh.append(t)
    diffs = sorted(b - a for a, b in zip(w1, wh))
    k = hi_reps - 1
    ns = float(np.median(diffs)) / k
    lo = diffs[len(diffs) // 4] / k
    hi = diffs[(3 * len(diffs)) // 4] / k
    return out, ns, {"w1": w1, "w4": wh, "q25": lo, "q75": hi}


# revision 29
# speedup vs baseline: 1.0785x; 1.0785x over previous
"""Trainium2 Bass kernel for nn_Attention_76450417868987.

Module: three Bahdanau-style additive attentions + gated fusion.
Sharding: pure data-parallel, batch 512 -> 64 per core across 8 cores.

v3 design (per core, heavy tensors host-cast to bf16):
  - Big tensors stream in natural layout [(b n), d] as bf16 wide tiles
    [128, A, 512] via three DMA rings: p_* on sync (HWDGE), feats on
    scalar (HWDGE), indicator/weight constants on gpsimd (SWDGE).
  - X = p + h_proj broadcast built on PE in PSUM [128, 1024] (2 banks):
    identity MM copies p; the two K=64 indicator MMs of a tile pair
    row-pack into disjoint PE row groups (ind64/hp duplicated into both
    partition halves) and run concurrently.
  - tanh on ScalarE over [128, 1024] PSUM -> bf16 SBUF (PSUM-source
    ACT is fast, ~225ns/op).
  - score col = ONE fused DVE scalar_tensor_tensor per tile:
    prod = (tt * 1.0) * awb, accum_out = row-sum -> scol_all [128, nt]
    f32, entirely on chip in flat layout. (tensor_tensor_reduce is not
    supported by this walrus build; scalar_tensor_tensor is.)
  - softmax WITHOUT max-subtraction (scores bounded by |aw|_1 ~ 11) and
    WITHOUT the scalar score bias (softmax-invariant). exp -> e_all.
  - block-diag einsum lhsT built in ONE DVE tensor_tensor per branch:
    lt = indT * broadcast(e) via a stride-0 view (DVE ops have a large
    per-op floor; avoid many small ops). No DRAM bounce.
  - einsum: per tile MM [128,64]x[128,512] accumulated into res PSUM,
    plus an N=1 MM with a ones column accumulating the softmax
    denominator (rides the already-loaded weights).
  - res normalized by reciprocal(den) at the end (DVE).
  - gate: batched PE transposes of cont/senti -> one PSUM tile + one
    copy, 20 bf16 matmuls + bias matmul, tanh, fused alpha-dot via
    scalar_tensor_tensor, sigmoid, blend.
"""

import os
import sys

if "/opt/trn_rl_repo" not in sys.path:
    sys.path.insert(0, "/opt/trn_rl_repo")

import numpy as np

B = 512
NA, NCP, NSW = 196, 50, 50
D = 512
M = 8
BL = B // M  # 64
NT_A = BL * NA // 128  # 98
NT_C = BL * NCP // 128  # 25
P = 128
A_A = 7  # wide-group size, att branch (98 = 14*7)
A_C = 5  # wide-group size, cpt/sw branches (25 = 5*5)

_CACHE = {}


def _segs(per_n):
    """Per flattened tile t of [BL*per_n, D]: list of (row_off, run_len, b)."""
    segs = []
    for t in range(BL * per_n // 128):
        lst = []
        r = t * 128
        while r < t * 128 + 128:
            b = r // per_n
            e = min((b + 1) * per_n, t * 128 + 128)
            lst.append((r - t * 128, e - r, b))
            r = e
        segs.append(lst)
    return segs


def _ind64_const(per_n):
    """[64, nt*128] bf16: ind[b, t*128+r] = 1 iff flat row t*128+r in batch b."""
    import ml_dtypes

    segs = _segs(per_n)
    nt = len(segs)
    a = np.zeros((BL, nt * 128), np.float32)
    for t, lst in enumerate(segs):
        for (off, ln, b) in lst:
            a[b, t * 128 + off : t * 128 + off + ln] = 1.0
    return a.astype(ml_dtypes.bfloat16)


def _indT_const(per_n):
    """[128, nt*64] bf16: indT[r, t*64+b] = 1 iff flat row t*128+r in batch b."""
    import ml_dtypes

    segs = _segs(per_n)
    nt = len(segs)
    a = np.zeros((P, nt * BL), np.float32)
    for t, lst in enumerate(segs):
        for (off, ln, b) in lst:
            a[off : off + ln, t * BL + b] = 1.0
    return a.astype(ml_dtypes.bfloat16)


def _build(nc, reps=1, mode="full"):
    import concourse.bass as bass  # noqa: F401
    from concourse import mybir
    from concourse.tile import TileContext

    f32 = mybir.dt.float32
    bf16 = mybir.dt.bfloat16
    AF = mybir.ActivationFunctionType
    OP = mybir.AluOpType
    AX = mybir.AxisListType

    def dpf(name, shape):
        return nc.declare_dram_parameter(name, shape, f32, isOutput=False)

    def dpb(name, shape):
        return nc.declare_dram_parameter(name, shape, bf16, isOutput=False)

    h_d = dpb("h", [BL, D])
    att_f = dpb("att_feats", [BL * NA, D])
    p_att = dpb("p_att_feats", [BL * NA, D])
    cpt_f = dpb("cpt_feats", [BL * NCP, D])
    p_cpt = dpb("p_cpt_feats", [BL * NCP, D])
    senti_d = dpf("senti_feats", [BL, D])
    sw_f = dpb("senti_word_feats", [BL * NSW, D])
    p_sw = dpb("p_senti_word_feats", [BL * NSW, D])

    w_h2att = dpb("c_h2att_w", [D, D])
    b_h2att = dpf("c_h2att_b", [1, D])
    w_h2cpt = dpb("c_h2cpt_w", [D, D])
    b_h2cpt = dpf("c_h2cpt_b", [1, D])
    aw_att_d = dpf("c_attA_w", [1, D])
    aw_cpt_d = dpf("c_cptA_w", [1, D])
    w_h2sw = dpb("s_h2word_w", [D, D])
    b_h2sw = dpf("s_h2word_b", [1, D])
    aw_sw_d = dpf("s_wordA_w", [1, D])
    w_th = dpb("t_h2att_w", [D, D])
    b_th = dpf("t_h2att_b", [1, D])
    w_tc = dpb("t_cont_w", [2 * D, D])
    b_tc = dpf("t_cont_b", [1, D])
    w_ts = dpb("t_senti_w", [2 * D, D])
    b_ts = dpf("t_senti_b", [1, D])
    w_ta_d = dpf("t_alpha_w", [1, D])
    b_ta_d = dpf("t_alpha_b", [1, 1])

    identbf_d = dpb("identbf", [P, P])
    ones4_d = dpf("ones4", [4, P])
    onescol_d = dpb("onescol", [P, 1])
    # ind64 duplicated into both partition halves so pairs of K=64
    # indicator matmuls can row-pack into disjoint PE row groups
    ind64_att_d = dpb("ind64_att", [P, NT_A * 128])
    ind64_50_d = dpb("ind64_50", [P, NT_C * 128])
    indT_att_d = dpb("indT_att", [P, NT_A * BL])
    indT_50_d = dpb("indT_50", [P, NT_C * BL])

    out_d = nc.declare_dram_parameter("out", [BL, 2 * D], f32, isOutput=True)

    with TileContext(nc) as tc:
        with (
            tc.tile_pool(name="const", bufs=1) as constp,
            tc.tile_pool(name="pio", bufs=2) as piop,
            tc.tile_pool(name="fio", bufs=2) as fiop,
            tc.tile_pool(name="work", bufs=2) as workp,
            tc.tile_pool(name="tanh", bufs=3) as tanhp,
            tc.tile_pool(name="small", bufs=2) as smallp,
            tc.tile_pool(name="psx", bufs=2, space="PSUM") as psxp,
            tc.tile_pool(name="psres", bufs=2, space="PSUM") as psresp,
            tc.tile_pool(name="psden", bufs=1, space="PSUM") as psdenp,
            tc.tile_pool(name="psbf", bufs=1, space="PSUM") as psbfp,
        ):
            for _rep in range(reps):
                # ---------------- setup ----------------
                identbf = constp.tile([P, P], bf16, tag="identbf")
                nc.sync.dma_start(identbf[:], identbf_d[:])
                ones4 = constp.tile([4, P], f32, tag="ones4")
                nc.sync.dma_start(ones4[:], ones4_d[:])
                onescol = constp.tile([P, 1], bf16, tag="onescol")
                nc.sync.dma_start(onescol[:], onescol_d[:])
                h_sb = constp.tile([BL, D], bf16, tag="h_sb")
                nc.sync.dma_start(h_sb[:], h_d[:])
                ind64_a = constp.tile([P, NT_A * 128], bf16, tag="ind64_a")
                nc.gpsimd.dma_start(ind64_a[:], ind64_att_d[:])
                ind64_5 = constp.tile([P, NT_C * 128], bf16, tag="ind64_5")
                nc.gpsimd.dma_start(ind64_5[:], ind64_50_d[:])
                indT_a = constp.tile([P, NT_A * BL], bf16, tag="indT_a")
                nc.gpsimd.dma_start(indT_a[:], indT_att_d[:])
                indT_5 = constp.tile([P, NT_C * BL], bf16, tag="indT_5")
                nc.gpsimd.dma_start(indT_5[:], indT_50_d[:])

                # hT[:, c, :] = h[:, 128c:128(c+1)].T  (PE transposes batched
                # into one PSUM tile, single copy out)
                hT = constp.tile([P, 4, BL], bf16, tag="hT")
                tp4 = psbfp.tile([P, 8, BL], bf16, tag="tpbf")
                for c in range(4):
                    nc.tensor.transpose(
                        tp4[:, c, :], h_sb[:, c * P : (c + 1) * P], identbf[:BL, :BL]
                    )
                nc.scalar.copy(hT[:], tp4[:, :4, :])

                def bcast_row(dram_row, tag, dtype):
                    """-> sbuf [128, D] with every partition = the dram row."""
                    row = smallp.tile([1, D], f32, tag="brow")
                    nc.sync.dma_start(row[:], dram_row[:1, :])
                    ps = psxp.tile([P, 2 * D], f32, tag="xps")
                    nc.tensor.matmul(
                        ps[:, :D], ones4[:1, :], row[:], start=True, stop=True
                    )
                    sb = constp.tile([P, D], dtype, tag=tag)
                    nc.scalar.copy(sb[:], ps[:, :D])
                    return sb

                awb = {
                    "a": bcast_row(aw_att_d, "awb_a", bf16),
                    "c": bcast_row(aw_cpt_d, "awb_c", bf16),
                    "s": bcast_row(aw_sw_d, "awb_s", bf16),
                }
                alphab = bcast_row(w_ta_d, "alphab", f32)

                ab_sb = smallp.tile([1, 1], f32, tag="ab_sb")
                nc.sync.dma_start(ab_sb[:], b_ta_d[:])
                ps = psxp.tile([P, 2 * D], f32, tag="xps")
                nc.tensor.matmul(
                    ps[:BL, :1], ones4[:1, :BL], ab_sb[:], start=True, stop=True
                )
                ab_col = constp.tile([BL, 1], f32, tag="ab_col")
                nc.scalar.copy(ab_col[:], ps[:BL, :1])

                def proj(wd, bd, tag):
                    """hp = h @ W + b -> sbuf [64, 512] bf16."""
                    hp_ps = psxp.tile([P, 2 * D], f32, tag="xps")
                    wt4 = constp.tile([P, 4, D], bf16, tag=f"w_{tag}")
                    nc.gpsimd.dma_start(
                        wt4[:], wd[:].rearrange("(c p) d -> p c d", p=128)
                    )
                    for c in range(4):
                        nc.tensor.matmul(
                            hp_ps[:BL, :D], hT[:, c, :], wt4[:, c, :],
                            start=(c == 0), stop=False,
                        )
                    brow = smallp.tile([1, D], f32, tag="brow")
                    nc.sync.dma_start(brow[:], bd[:1, :])
                    nc.tensor.matmul(
                        hp_ps[:BL, :D], ones4[:1, :BL], brow[:], start=False, stop=True
                    )
                    # duplicated into both partition halves for row-packed MMs
                    sb = constp.tile([P, D], bf16, tag=tag)
                    nc.scalar.copy(sb[:BL, :], hp_ps[:BL, :D])
                    nc.scalar.copy(sb[BL:, :], hp_ps[:BL, :D])
                    return sb

                hp = {
                    "a": proj(w_h2att, b_h2att, "hp_a"),
                    "c": proj(w_h2cpt, b_h2cpt, "hp_c"),
                    "s": proj(w_h2sw, b_h2sw, "hp_s"),
                }

                # scol_all / e_all: flat scores for all 3 branches
                # columns [0:98]=a, [98:123]=c, [123:148]=s
                NT_ALL = NT_A + 2 * NT_C
                scol_all = constp.tile([P, NT_ALL], f32, tag="scol_all")
                e_all = constp.tile([P, NT_ALL], f32, tag="e_all")
                COFF = {"a": 0, "c": NT_A, "s": NT_A + NT_C}

                # ---------------- score phase ----------------
                def score_branch(key, p_dram, nt, ind_sb):
                    Ag = A_A if nt == NT_A else A_C
                    for g in range(nt // Ag):
                        wide7 = piop.tile([P, A_A, D], bf16, tag="p_in")
                        wide = wide7[:, :Ag, :]
                        view = p_dram[g * Ag * 128 : (g + 1) * Ag * 128, :].rearrange(
                            "(a p) d -> p a d", p=128
                        )
                        nc.sync.dma_start(wide, view)
                        if mode == "dma":
                            continue
                        # process pairs of tiles -> one [128, 1024] PSUM tile.
                        # identity MMs use the full array (K=128); the two
                        # K=64 indicator MMs row-pack into disjoint row
                        # groups (partitions 0:64 / 64:128) and run
                        # concurrently on the PE.
                        pairs = [(i, min(i + 2, Ag)) for i in range(0, Ag, 2)]
                        for (i0, i1) in pairs:
                            na = i1 - i0
                            xps = psxp.tile([P, 2 * D], f32, tag="xps")
                            for a in range(i0, i1):
                                col = (a - i0) * D
                                nc.tensor.matmul(
                                    xps[:, col : col + D],
                                    identbf[:],
                                    wide[:, a, :],
                                    start=True,
                                    stop=False,
                                )
                            for a in range(i0, i1):
                                t = g * Ag + a
                                col = (a - i0) * D
                                rg = (a - i0) * BL  # row group 0 or 64
                                nc.tensor.matmul(
                                    xps[:, col : col + D],
                                    ind_sb[rg : rg + BL, t * 128 : (t + 1) * 128],
                                    hp[key][rg : rg + BL, :],
                                    start=False,
                                    stop=True,
                                )
                            if mode == "pe":
                                continue
                            tt = tanhp.tile([P, 2 * D], bf16, tag="tt")
                            nc.scalar.activation(
                                tt[:, : na * D], xps[:, : na * D], AF.Tanh
                            )
                            if mode == "act":
                                continue
                            prod = tanhp.tile([P, 2 * D], bf16, tag="prod")
                            for a in range(i0, i1):
                                t = g * Ag + a
                                col = (a - i0) * D
                                # fused: prod = tt * awb, scol = sum(prod)
                                nc.vector.scalar_tensor_tensor(
                                    prod[:, col : col + D],
                                    tt[:, col : col + D],
                                    1.0,
                                    awb[key][:],
                                    OP.mult,
                                    OP.mult,
                                    accum_out=scol_all[
                                        :, COFF[key] + t : COFF[key] + t + 1
                                    ],
                                )

                def build_lhst(key, nt, indT_sb):
                    """exp of scores -> block-diag lhsT stack [128, nt*64] bf16."""
                    c0 = COFF[key]
                    lt = constp.tile([P, nt * BL], bf16, tag=f"lt_{key}")
                    if mode in ("dma", "pe", "act"):
                        return indT_sb  # use raw indicator so einsum still runs
                    nc.scalar.activation(
                        e_all[:, c0 : c0 + nt], scol_all[:, c0 : c0 + nt], AF.Exp
                    )
                    # ONE broadcast tensor_tensor per branch: lt[r, (t,b)] =
                    # indT[r, (t,b)] * e[r, t] via a stride-0 view of e.
                    e_bc = (
                        e_all[:, c0 : c0 + nt]
                        .rearrange("p t -> p t ()")
                        .broadcast_to((P, nt, BL))
                    )
                    nc.vector.tensor_tensor(
                        lt[:].rearrange("p (t b) -> p t b", b=BL),
                        indT_sb[:].rearrange("p (t b) -> p t b", b=BL),
                        e_bc,
                        op=OP.mult,
                    )
                    return lt

                # ---------------- einsum phase ----------------
                den_all = psdenp.tile([BL, 16], f32, tag="den")
                DCOL = {"a": 0, "c": 4, "s": 8}

                def einsum_branch(key, f_dram, lt, nt):
                    res = psresp.tile([BL, D], f32, tag="res")
                    dc = DCOL[key]
                    Ag = A_A if nt == NT_A else A_C
                    for g in range(nt // Ag):
                        wide7 = fiop.tile([P, A_A, D], bf16, tag="f_in")
                        wide = wide7[:, :Ag, :]
                        view = f_dram[g * Ag * 128 : (g + 1) * Ag * 128, :].rearrange(
                            "(a p) d -> p a d", p=128
                        )
                        nc.scalar.dma_start(wide, view)
                        if mode == "dma":
                            continue
                        for a in range(Ag):
                            t = g * Ag + a
                            nc.tensor.matmul(
                                res[:],
                                lt[:, t * BL : (t + 1) * BL],
                                wide[:, a, :],
                                start=(t == 0),
                                stop=(t == nt - 1),
                            )
                            # denominator rides the already-loaded weights
                            nc.tensor.matmul(
                                den_all[:, dc : dc + 1],
                                lt[:, t * BL : (t + 1) * BL],
                                onescol[:],
                                start=(t == 0),
                                stop=(t == nt - 1),
                            )
                    return res

                cont = constp.tile([BL, 2 * D], f32, tag="cont")
                sent = constp.tile([BL, 2 * D], f32, tag="sent")
                nc.sync.dma_start(sent[:, :D], senti_d[:])

                full = mode == "full"

                def norm_res(key, res, dst):
                    """dst = res / den (softmax denominator)."""
                    dc = DCOL[key]
                    rec = smallp.tile([BL, 1], f32, tag=f"rec_{key}")
                    nc.vector.reciprocal(rec[:], den_all[:, dc : dc + 1])
                    nc.vector.tensor_scalar_mul(dst, res[:], rec[:])

                # emission order tuned so each branch's exp/lhsT-build lands
                # in the ACT/DVE queues right after that branch's tanh/stt
                # ops (no FIFO stall behind the next branch), while score(c)
                # keeps the PE busy while lhsT(a) is being built.
                score_branch("a", p_att, NT_A, ind64_a)
                lt_a = build_lhst("a", NT_A, indT_a)
                score_branch("c", p_cpt, NT_C, ind64_5)
                res_a = einsum_branch("a", att_f, lt_a, NT_A)
                if full:
                    norm_res("a", res_a, cont[:, :D])
                lt_c = build_lhst("c", NT_C, indT_5)
                score_branch("s", p_sw, NT_C, ind64_5)
                res_c = einsum_branch("c", cpt_f, lt_c, NT_C)
                if full:
                    norm_res("c", res_c, cont[:, D:])
                lt_s = build_lhst("s", NT_C, indT_5)
                res_s = einsum_branch("s", sw_f, lt_s, NT_C)
                if full:
                    norm_res("s", res_s, sent[:, D:])

                if not full:
                    fin0 = constp.tile([BL, 2 * D], f32, tag="fin")
                    nc.vector.memset(fin0[:], 0.0)
                    nc.sync.dma_start(out_d[:], fin0[:])
                    continue

                # ---------------- gate ----------------
                # cast cont/sent to bf16, batch 8 PE transposes into one PSUM
                # tile, single ACT copy out -> lhsT chunks [128, 8, 64]
                def gate_lhsT(src_sb, tag):
                    cb = workp.tile([BL, 2 * D], bf16, tag="gcast")
                    nc.vector.tensor_copy(cb[:], src_sb[:])
                    tp = psbfp.tile([P, 8, BL], bf16, tag="tpbf")
                    for c in range(8):
                        nc.tensor.transpose(
                            tp[:, c, :], cb[:, c * P : (c + 1) * P],
                            identbf[:BL, :BL],
                        )
                    ct = constp.tile([P, 8, BL], bf16, tag=tag)
                    nc.scalar.copy(ct[:], tp[:])
                    return ct

                contT = gate_lhsT(cont, "contT")
                sentT = gate_lhsT(sent, "sentT")

                wgtc = constp.tile([P, 8, D], bf16, tag="wgtc")
                nc.gpsimd.dma_start(
                    wgtc[:], w_tc[:].rearrange("(c p) d -> p c d", p=128)
                )
                wgts = constp.tile([P, 8, D], bf16, tag="wgts")
                nc.gpsimd.dma_start(
                    wgts[:], w_ts[:].rearrange("(c p) d -> p c d", p=128)
                )
                wgth = constp.tile([P, 4, D], bf16, tag="wgth")
                nc.gpsimd.dma_start(
                    wgth[:], w_th[:].rearrange("(c p) d -> p c d", p=128)
                )
                g_ps = psresp.tile([BL, D], f32, tag="res")
                first = True
                for (src_t, wt4, nch) in (
                    (contT, wgtc, 8),
                    (sentT, wgts, 8),
                    (hT, wgth, 4),
                ):
                    for c in range(nch):
                        nc.tensor.matmul(
                            g_ps[:], src_t[:, c, :], wt4[:, c, :], start=first,
                            stop=False,
                        )
                        first = False
                b3 = smallp.tile([3, D], f32, tag="b3")
                nc.sync.dma_start(b3[0:1, :], b_tc[:1, :])
                nc.sync.dma_start(b3[1:2, :], b_ts[:1, :])
                nc.sync.dma_start(b3[2:3, :], b_th[:1, :])
                nc.tensor.matmul(
                    g_ps[:], ones4[:3, :BL], b3[:], start=False, stop=True
                )

                g_sb = workp.tile([BL, D], f32, tag="g_sb")
                nc.scalar.activation(g_sb[:], g_ps[:], AF.Tanh)
                gdump = workp.tile([BL, D], f32, tag="gdump")
                gacc = smallp.tile([BL, 1], f32, tag="gacc")
                nc.vector.scalar_tensor_tensor(
                    gdump[:], g_sb[:], 1.0, alphab[:BL, :], OP.mult, OP.mult,
                    accum_out=gacc[:],
                )
                gate = smallp.tile([BL, 1], f32, tag="gate")
                nc.scalar.activation(gate[:], gacc[:], AF.Sigmoid, bias=ab_col[:])

                diff = constp.tile([BL, 2 * D], f32, tag="diff")
                nc.vector.tensor_sub(diff[:], cont[:], sent[:])
                prd = constp.tile([BL, 2 * D], f32, tag="prd")
                nc.vector.tensor_scalar_mul(prd[:], diff[:], gate[:, 0:1])
                fin = constp.tile([BL, 2 * D], f32, tag="fin")
                nc.vector.tensor_add(fin[:], sent[:], prd[:])
                nc.sync.dma_start(out_d[:], fin[:])

    return nc


def _fixup_multiwait(nc):
    """This walrus build allows only ONE sync wait per instruction (except
    InstEventSemaphore). Split extra waits onto same-engine NOPs in front."""
    from concourse import mybir

    nfix = 0
    for fn in nc.m.functions:
        for blk in fn.blocks:
            new = []
            for inst in blk.instructions:
                si = inst.sync_info
                waits = list(si.on_wait) if si is not None else []
                if len(waits) > 1 and type(inst).__name__ != "InstEventSemaphore":
                    for w in waits[:-1]:
                        nop = mybir.InstNoOp(
                            name=nc.get_next_instruction_name(), ins=[], outs=[]
                        )
                        nop.engine = inst.engine
                        nop.sync_info = mybir.SyncInfo(on_wait=[w], on_update=[])
                        nc.register_instruction(nop)
                        new.append(nop)
                        nfix += 1
                    si.on_wait = waits[-1:]
                new.append(inst)
            blk.instructions[:] = new
    return nfix


def _get_nc(reps=1, mode="full"):
    key = f"nc{reps}_{mode}"
    if key not in _CACHE:
        import concourse.bass as bass

        nc = bass.Bass()
        _build(nc, reps=reps, mode=mode)
        nc.finalize()
        _fixup_multiwait(nc)
        _CACHE[key] = nc
    return _CACHE[key]


def _make_in_maps(inputs):
    import ml_dtypes

    bf = ml_dtypes.bfloat16
    f = lambda x: np.ascontiguousarray(np.asarray(x), dtype=np.float32)
    fb = lambda x: np.ascontiguousarray(
        np.asarray(x, dtype=np.float32).astype(bf)
    )

    consts = {
        "identbf": np.eye(P, dtype=np.float32).astype(bf),
        "onescol": np.ones((P, 1), np.float32).astype(bf),
        "ones4": np.ones((4, P), np.float32),
        # duplicated into both partition halves for row-packed PE MMs
        "ind64_att": np.concatenate([_ind64_const(NA)] * 2, axis=0),
        "ind64_50": np.concatenate([_ind64_const(NCP)] * 2, axis=0),
        "indT_att": _indT_const(NA),
        "indT_50": _indT_const(NCP),
    }
    weights = {
        "c_h2att_w": fb(inputs["c_h2att_w"]),
        "c_h2att_b": f(inputs["c_h2att_b"]).reshape(1, D),
        "c_h2cpt_w": fb(inputs["c_h2cpt_w"]),
        "c_h2cpt_b": f(inputs["c_h2cpt_b"]).reshape(1, D),
        "c_attA_w": f(inputs["c_attA_w"]).reshape(1, D),
        "c_cptA_w": f(inputs["c_cptA_w"]).reshape(1, D),
        "s_h2word_w": fb(inputs["s_h2word_w"]),
        "s_h2word_b": f(inputs["s_h2word_b"]).reshape(1, D),
        "s_wordA_w": f(inputs["s_wordA_w"]).reshape(1, D),
        "t_h2att_w": fb(inputs["t_h2att_w"]),
        "t_h2att_b": f(inputs["t_h2att_b"]).reshape(1, D),
        "t_cont_w": fb(inputs["t_cont_w"]),
        "t_cont_b": f(inputs["t_cont_b"]).reshape(1, D),
        "t_senti_w": fb(inputs["t_senti_w"]),
        "t_senti_b": f(inputs["t_senti_b"]).reshape(1, D),
        "t_alpha_w": f(inputs["t_alpha_w"]).reshape(1, D),
        "t_alpha_b": f(inputs["t_alpha_b"]).reshape(1, 1),
    }
    in_maps = []
    for i in range(M):
        sl = slice(i * BL, (i + 1) * BL)
        m = {
            "h": fb(inputs["h"][sl]),
            "att_feats": fb(inputs["att_feats"][sl]).reshape(BL * NA, D),
            "p_att_feats": fb(inputs["p_att_feats"][sl]).reshape(BL * NA, D),
            "cpt_feats": fb(inputs["cpt_feats"][sl]).reshape(BL * NCP, D),
            "p_cpt_feats": fb(inputs["p_cpt_feats"][sl]).reshape(BL * NCP, D),
            "senti_feats": f(inputs["senti_feats"][sl]),
            "senti_word_feats": fb(inputs["senti_word_feats"][sl]).reshape(
                BL * NSW, D
            ),
            "p_senti_word_feats": fb(inputs["p_senti_word_feats"][sl]).reshape(
                BL * NSW, D
            ),
        }
        m.update(weights)
        m.update(consts)
        in_maps.append(m)
    return in_maps


def _run(inputs, trace=False):
    from concourse.bass_utils import run_bass_kernel_spmd

    nc = _get_nc()
    in_maps = _make_in_maps(inputs)
    r = run_bass_kernel_spmd(nc, in_maps, core_ids=list(range(M)), trace=trace)
    out = np.concatenate([r.results[i]["out"] for i in range(M)], axis=0)
    return out, r


def kernel(**inputs):
    out, _ = _run(inputs, trace=False)
    return out


def _timed_runner_make(nc, in_maps, iters):
    """Build a runner for nc with device-resident inputs and pre-staged
    donated output buffers; returns run(i) -> (wall_ns, out_np)."""
    import time

    import jax
    from jax.sharding import Mesh, NamedSharding, PartitionSpec

    try:
        from jax.experimental.shard_map import shard_map
    except ImportError:
        from jax.shard_map import shard_map

    from concourse import bass2jax, mybir
    from concourse.bass2jax import _bass_exec_p

    bass2jax.install_neuronx_cc_hook()
    partition_name = nc.partition_id_tensor.name if nc.partition_id_tensor else None

    in_names, out_names, out_avals, zero_outs = [], [], [], []
    for alloc in nc.m.functions[0].allocations:
        if not isinstance(alloc, mybir.MemoryLocationSet):
            continue
        name = alloc.memorylocations[0].name
        if alloc.kind == "ExternalInput":
            if name != partition_name:
                in_names.append(name)
        elif alloc.kind == "ExternalOutput":
            out_names.append(name)
            out_avals.append(
                jax.core.ShapedArray(
                    tuple(alloc.tensor_shape), mybir.dt.np(alloc.dtype)
                )
            )
            zero_outs.append(
                np.zeros(tuple(alloc.tensor_shape), mybir.dt.np(alloc.dtype))
            )
    n_params = len(in_names)
    n_outs = len(out_names)
    all_in = list(in_names) + list(out_names)
    if partition_name:
        all_in.append(partition_name)

    def _body(*args):
        operands = list(args)
        if partition_name:
            operands.append(bass2jax.partition_id_tensor())
        return tuple(
            _bass_exec_p.bind(
                *operands,
                out_avals=tuple(out_avals),
                in_names=tuple(all_in),
                out_names=tuple(out_names),
                lowering_input_output_aliases=(),
                sim_require_finite=False,
                sim_require_nnan=False,
                nc=nc,
            )
        )

    devices = jax.devices()[:M]
    mesh = Mesh(np.asarray(devices), ("core",))
    donate = tuple(range(n_params, n_params + n_outs))
    sharded = jax.jit(
        shard_map(
            _body,
            mesh=mesh,
            in_specs=(PartitionSpec("core"),) * (n_params + n_outs),
            out_specs=(PartitionSpec("core"),) * n_outs,
            check_rep=False,
        ),
        donate_argnums=donate,
        keep_unused=True,
    )
    sh = NamedSharding(mesh, PartitionSpec("core"))
    per_core = [[np.asarray(m[name]) for name in in_names] for m in in_maps]
    args = [
        jax.device_put(
            np.concatenate([per_core[c][i] for c in range(M)], axis=0), sh
        )
        for i in range(n_params)
    ]
    # one donated zero-output set per call, staged up front
    zsets = []
    for _ in range(iters + 1):
        zsets.append(
            [jax.device_put(np.concatenate([z] * M, axis=0), sh) for z in zero_outs]
        )
    out = sharded(*args, *zsets[-1])
    jax.block_until_ready(out)

    def run(i):
        t0 = time.perf_counter()
        o = sharded(*args, *zsets[i])
        jax.block_until_ready(o)
        t1 = time.perf_counter()
        return (t1 - t0) * 1e9, np.asarray(o[0])

    return run


def profile(inputs, iters=14, mode="full", hi_reps=16):
    """Interleave V1/V<hi> executions; per-pair diffs cancel slow drift of
    the axon dispatch round-trip: T = median(w_hi_i - w1_i) / (hi-1)."""
    in_maps = _make_in_maps(inputs)
    r1 = _timed_runner_make(_get_nc(1, mode), in_maps, iters)
    rh = _timed_runner_make(_get_nc(hi_reps, mode), in_maps, iters)
    w1, wh = [], []
    out = None
    for i in range(iters):
        t, out = r1(i)
        w1.append(t)
        t, _ = rh(i)
        wh.append(t)
    diffs = sorted(b - a for a, b in zip(w1, wh))
    k = hi_reps - 1
    ns = float(np.median(diffs)) / k
    lo = diffs[len(diffs) // 4] / k
    hi = diffs[(3 * len(diffs)) // 4] / k
    return out, ns, {"w1": w1, "w4": wh, "q25": lo, "q75": hi}
